# revision 22
# baseline (speedup 1.0000x reference)
"""Trainium2 Bass kernel for causal multi-head attention with RoPE.

Model: B=2, S=2048, H=2048, 16 heads x 128 head-dim.
  qkv = x @ w_qkv.T ; RoPE(q, k); causal softmax(q k^T / sqrt(dh)) @ v; out = attn @ w_o.T

Sharding: tensor-parallel over heads. Each of the 8 cores owns 2 heads:
it computes q/k/v projections for its heads (w_qkv row slices), runs
flash-style causal attention for them, and applies its slice of w_o
columns, producing a partial [B,S,H] output. The host sums the 8
partials in fp32 (the all-reduce "unshard" of the TP strategy).

On-core layout choices:
  - Q,K projected in natural [token, dim] layout so RoPE's rotate-half
    pairs sit at free-dim offsets (cross-partition DVE reads are
    rejected by the walrus verifier), then PE-transposed to [dim, token]
    for the scores matmul.
  - Scores computed transposed (S^T[kt, qt]) so the exp'd probabilities
    feed the PV matmul directly with no per-block transpose; the softmax
    denominator comes from an M=1 ones-matmul accumulated in PSUM, and
    is folded in after PV via reciprocal + gpsimd partition_broadcast.
  - No max-subtraction in softmax: inputs are unit-scale gaussians, so
    scaled scores are O(10) and exp stays comfortably inside fp32/bf16.
  - All matmuls in bf16 with fp32 PSUM accumulation.
"""

import contextlib
import math

import numpy as np
import ml_dtypes

B = 2
S = 2048
HID = 2048
NH = 16
DH = 128
NCORES = 8
HPC = NH // NCORES  # heads per core
CH = 512            # chunk (free-dim) size
NEG = -1.0e30

_STATE = {}


# ----------------------------------------------------------------------------
# device kernel
# ----------------------------------------------------------------------------

def _emit_body(nc, r, seq_len):
    """Emit one full pass of the computation. `r` holds pools + consts."""
    import concourse.bass_isa as bass_isa
    import concourse.mybir as mybir

    bf16 = mybir.dt.bfloat16
    f32 = mybir.dt.float32
    Exp = mybir.ActivationFunctionType.Exp
    NT = seq_len // 128
    TC = seq_len // CH
    NHB = HID // 128
    SCALE = 1.0 / math.sqrt(DH)

    def oproj_fillers(b, qi, at_pair):
        """Emitters for chunk qi's output projection, one (tt, oc) tile
        each — interleaved into the next chunk's attention loop as PE
        filler so exp latency on ACT never stalls the PE queue."""
        tiles = {}

        def make(tt, oc):
            def emit():
                if tt not in tiles:
                    tiles[tt] = r.opool.tile([128, HID], bf16, tag="ot",
                                             name="ot")
                ot = tiles[tt]
                pop = r.psC.tile([128, CH], f32, tag="C", name="pop")
                for h in range(2):
                    nc.tensor.matmul(
                        pop[:],
                        at_pair[h][:, tt * 128:(tt + 1) * 128],
                        r.wo_sb[:, h, oc * CH:(oc + 1) * CH],
                        start=(h == 0), stop=(h == 1),
                    )
                nc.vector.tensor_copy(ot[:, oc * CH:(oc + 1) * CH], pop[:])
                if oc == HID // CH - 1:
                    nc.sync.dma_start(
                        r.out_d[b, qi * CH + tt * 128:
                                qi * CH + (tt + 1) * 128, :],
                        ot[:],
                    )
            return emit

        return [make(tt, oc) for tt in range(4) for oc in range(HID // CH)]

    for b in range(B):
        # ---------------- QKV projection + RoPE ----------------
        qks = []
        for nm in ("q0t", "q1t", "k0t", "k1t"):
            qks.append(r.qkpool.tile([128, seq_len], bf16, tag="qkt", name=nm))
        q0t, q1t, k0t, k1t = qks
        vt = r.vpool.tile([128, NT, 2 * DH], bf16, tag="vt", name="vt")
        xTb = r.xT[b].rearrange("(n p) t -> p n t", p=128)

        def emit_transposes(qr, j):
            for o, dst in enumerate((q0t, q1t, k0t, k1t)):
                ptr = r.psC.tile([128, 128], bf16, tag="C", name="ptr")
                nc.tensor.transpose(
                    ptr[:], qr[:, o * 128:(o + 1) * 128], r.ident[:])
                nc.vector.tensor_copy(
                    dst[:, j * 128:(j + 1) * 128], ptr[:])

        pending = None  # transposes run one tile behind their RoPE chain
        for tc4 in range(TC):
            xts = r.xpool.tile([128, NHB, CH], bf16, tag="xt", name="xts")
            nc.sync.dma_start(
                xts[:], xTb[:, :, tc4 * CH:(tc4 + 1) * CH])
            for tt in range(4):
                j = 4 * tc4 + tt
                psqk = r.psA.tile([128, 4 * DH], f32, tag="A", name="psqk")
                psv = r.psB.tile([128, 2 * DH], f32, tag="B", name="psv")
                for hb in range(NHB):
                    lhs = xts[:, hb, tt * 128:(tt + 1) * 128]
                    nc.tensor.matmul(
                        psqk[:], lhs, r.wqk_sb[:, hb, :],
                        start=(hb == 0), stop=(hb == NHB - 1),
                    )
                    nc.tensor.matmul(
                        psv[:], lhs, r.wv_sb[:, hb, :],
                        start=(hb == 0), stop=(hb == NHB - 1),
                    )
                if pending is not None:
                    emit_transposes(*pending)
                nc.scalar.copy(vt[:, j, :], psv[:])
                # RoPE on the whole [128, 512] q0|q1|k0|k1 block at once:
                # strided views pair (d, d+64) within each 128-block.
                ps4 = psqk.rearrange("p (o h d) -> p o h d", h=2, d=64)
                sin4 = r.sinc[:, j, :].rearrange("p (o h d) -> p o h d", h=2, d=64)
                t1 = r.tpool.tile([128, CH], f32, tag="t1", name="t1")
                t14 = t1.rearrange("p (o h d) -> p o h d", h=2, d=64)
                t2 = r.tpool.tile([128, CH], f32, tag="t2", name="t2")
                nc.vector.tensor_mul(t14[:, :, 0, :], ps4[:, :, 1, :],
                                     sin4[:, :, 0, :])
                nc.vector.tensor_mul(t14[:, :, 1, :], ps4[:, :, 0, :],
                                     sin4[:, :, 1, :])
                nc.vector.tensor_mul(t2[:], psqk[:], r.cosc[:, j, :])
                qr = r.qrpool.tile([128, CH], bf16, tag="qr", name="qr")
                nc.vector.tensor_add(qr[:], t1[:], t2[:])
                pending = (qr, j)
        emit_transposes(*pending)

        # ---------------- attention + output projection --------
        prev_at = None
        fillers = []
        for qi in range(TC):
            if prev_at is not None:
                fillers = oproj_fillers(b, qi - 1, prev_at)
            at_pair = []
            for h, (Q, K) in enumerate(((q0t, k0t), (q1t, k1t))):
                pso = r.psB.tile([128, CH], f32, tag="B", name="pso")
                racc = r.rspool.tile([1, CH], f32, tag="racc", name="racc")
                nj = 4 * qi + 4

                def emit_scores(jb):
                    # Diagonal blocks only produce nonzero probabilities for
                    # qt >= kt; narrow all work to that column subrange.
                    r8 = jb - 4 * qi
                    lo = 128 * r8 if r8 > 0 else 0
                    sub = slice(lo, CH)
                    pss = r.psA.tile([128, CH], f32, tag="A", name="pss")
                    nc.tensor.matmul(
                        pss[:, sub], K[:, jb * 128:(jb + 1) * 128],
                        Q[:, qi * CH + lo:(qi + 1) * CH],
                        start=True, stop=True,
                    )
                    if r8 >= 0:
                        nc.vector.tensor_add(
                            pss[:, sub], pss[:, sub], r.mask_sb[:, r8, sub])
                    pt = r.ptpool.tile([128, CH], bf16, tag="pt", name="pt")
                    nc.scalar.activation(pt[:, sub], pss[:, sub], Exp,
                                         scale=SCALE)
                    return pt, lo

                def emit_pv(jb, pt, lo):
                    sub = slice(lo, CH)
                    nc.tensor.matmul(
                        pso[:, sub], vt[:, jb, h * DH:(h + 1) * DH], pt[:, sub],
                        start=(jb == 0), stop=(jb == nj - 1))
                    # softmax denominator on the (otherwise idle) GPSIMD:
                    # per-block partition sum, accumulated along the free dim
                    # of a [1, CH] sbuf tile by DVE.
                    prd = r.rspool.tile([128, CH], f32, tag="prd", name="prd")
                    nc.gpsimd.partition_all_reduce(
                        prd[:, sub], pt[:, sub], 128, bass_isa.ReduceOp.add)
                    if jb == 0:
                        nc.vector.tensor_copy(racc[:], prd[0:1, :])
                    else:
                        nc.vector.tensor_add(
                            racc[0:1, sub], racc[0:1, sub], prd[0:1, sub])

                # scores run two blocks ahead of PV so the PE never waits
                # on the mask+exp chain of the block it just scored; oproj
                # matmuls from the previous chunk fill remaining PE slack.
                SKEW = 2
                queue = []
                for jb in range(nj):
                    queue.append((jb, emit_scores(jb)))
                    if fillers:
                        fillers.pop(0)()
                    if len(queue) > SKEW:
                        pj, args = queue.pop(0)
                        emit_pv(pj, *args)
                for pj, args in queue:
                    emit_pv(pj, *args)

                rs = r.rspool.tile([1, CH], f32, tag="rs", name="rs")
                nc.vector.reciprocal(rs[:], racc[:])
                rsb = r.rspool.tile([128, CH], f32, tag="rsb", name="rsb")
                nc.gpsimd.partition_broadcast(rsb[:], rs[:])
                at = r.atpool.tile([128, CH], bf16, tag="at", name="at")
                nc.vector.tensor_mul(at[:], pso[:], rsb[:])
                at_pair.append(at)
            for f in fillers:
                f()
            fillers = []
            prev_at = at_pair
        for f in oproj_fillers(b, TC - 1, prev_at):
            f()


class _Res:
    pass


def build_nc(seq_len=S, loop_n=1):
    """Build the per-core program. loop_n>1 wraps the body in a hardware
    loop — a timing-only variant used to measure per-iteration device
    time through the noisy dispatch path."""
    import concourse.mybir as mybir
    import concourse.tile as tile
    from concourse import bacc
    from concourse.masks import make_identity

    bf16 = mybir.dt.bfloat16
    f32 = mybir.dt.float32
    NT = seq_len // 128

    nc = bacc.Bacc("TRN2", target_bir_lowering=False, debug=False)

    r = _Res()
    r.xT = nc.dram_tensor("xt", [B, HID, seq_len], bf16, kind="ExternalInput")
    wqk = nc.dram_tensor("wqk", [HID, 4 * DH], bf16, kind="ExternalInput")
    wv = nc.dram_tensor("wv", [HID, 2 * DH], bf16, kind="ExternalInput")
    wo = nc.dram_tensor("wo", [2 * DH, HID], bf16, kind="ExternalInput")
    rope_d = {}
    for nm in ("cosc", "sinc"):
        rope_d[nm] = nc.dram_tensor(nm, [seq_len, 4 * DH], bf16,
                                    kind="ExternalInput")
    mask_d = nc.dram_tensor("masks", [4, 128, CH], f32, kind="ExternalInput")
    r.out_d = nc.dram_tensor("out", [B, seq_len, HID], bf16,
                             kind="ExternalOutput")

    with tile.TileContext(nc) as tc:
        with (
            tc.tile_pool(name="consts", bufs=1) as cpool,
            tc.tile_pool(name="x", bufs=2) as xpool,
            tc.tile_pool(name="qk", bufs=8) as qkpool,
            tc.tile_pool(name="v", bufs=2) as vpool,
            tc.tile_pool(name="pt", bufs=4) as ptpool,
            tc.tile_pool(name="at", bufs=4) as atpool,
            tc.tile_pool(name="tmp", bufs=2) as tpool,
            tc.tile_pool(name="qr", bufs=3) as qrpool,
            tc.tile_pool(name="rs", bufs=2) as rspool,
            tc.tile_pool(name="o", bufs=2) as opool,
            tc.tile_pool(name="psA", bufs=4, space="PSUM") as psA,
            tc.tile_pool(name="psB", bufs=2, space="PSUM") as psB,
            tc.tile_pool(name="psC", bufs=2, space="PSUM") as psC,
        ):
            r.xpool, r.qkpool, r.vpool, r.ptpool = xpool, qkpool, vpool, ptpool
            r.atpool, r.tpool, r.qrpool, r.rspool = atpool, tpool, qrpool, rspool
            r.opool = opool
            r.psA, r.psB, r.psC = psA, psB, psC

            r.wqk_sb = cpool.tile([128, HID // 128, 4 * DH], bf16, name="wqk_sb")
            nc.sync.dma_start(r.wqk_sb[:], wqk.rearrange("(n p) o -> p n o", p=128))
            r.wv_sb = cpool.tile([128, HID // 128, 2 * DH], bf16, name="wv_sb")
            nc.sync.dma_start(r.wv_sb[:], wv.rearrange("(n p) o -> p n o", p=128))
            r.wo_sb = cpool.tile([128, 2, HID], bf16, name="wo_sb")
            nc.sync.dma_start(r.wo_sb[:], wo.rearrange("(n p) o -> p n o", p=128))
            for nm in ("cosc", "sinc"):
                t = cpool.tile([128, NT, 4 * DH], bf16, name=nm)
                nc.sync.dma_start(t[:], rope_d[nm].rearrange("(n p) d -> p n d", p=128))
                setattr(r, nm, t)
            r.mask_sb = cpool.tile([128, 4, CH], f32, name="mask_sb")
            nc.sync.dma_start(r.mask_sb[:], mask_d.rearrange("n p o -> p n o"))
            r.ident = cpool.tile([128, 128], bf16, name="ident")
            make_identity(nc, r.ident[:])

            loop_ctx = (tc.For_i(0, loop_n, 1) if loop_n > 1
                        else contextlib.nullcontext())
            with loop_ctx:
                _emit_body(nc, r, seq_len)

    nc.compile()
    return nc


# ----------------------------------------------------------------------------
# host-side sharding / tables
# ----------------------------------------------------------------------------

def host_tables(seq_len=S):
    bf = ml_dtypes.bfloat16
    inv = 1.0 / (10000.0 ** (np.arange(0, DH, 2, dtype=np.float64) / DH))
    ang = np.arange(seq_len, dtype=np.float64)[:, None] * inv[None, :]  # [S, 64]
    cos = np.cos(ang)
    sin = np.sin(ang)
    cos_td = np.concatenate([cos, cos], axis=1)                  # [S, 128]
    ssin_td = np.concatenate([-sin, sin], axis=1)                # signed swap mult
    tabs = {
        "cosc": np.ascontiguousarray(np.tile(cos_td, (1, 4))).astype(bf),
        "sinc": np.ascontiguousarray(np.tile(ssin_td, (1, 4))).astype(bf),
    }
    p = np.arange(128)[:, None]
    f = np.arange(CH)[None, :]
    masks = np.stack(
        [np.where(p + 128 * ri <= f, 0.0, NEG) for ri in range(4)]
    ).astype(np.float32)
    tabs["masks"] = masks
    return tabs


def host_in_maps(x, w_qkv, w_o, seq_len=S):
    bf = ml_dtypes.bfloat16
    x = np.asarray(x, dtype=np.float32)
    w_qkv = np.asarray(w_qkv, dtype=np.float32)
    w_o = np.asarray(w_o, dtype=np.float32)
    xT = np.ascontiguousarray(x.transpose(0, 2, 1)).astype(bf)
    tabs = host_tables(seq_len)
    maps = []
    for c in range(NCORES):
        h0 = HPC * c
        rows = []
        for base in (0, HID):  # q rows, then k rows
            for h in range(h0, h0 + HPC):
                rows.append(w_qkv[base + h * DH:base + (h + 1) * DH])
        wqk_c = np.ascontiguousarray(np.concatenate(rows, axis=0).T).astype(bf)
        vrows = [w_qkv[2 * HID + h * DH:2 * HID + (h + 1) * DH]
                 for h in range(h0, h0 + HPC)]
        wv_c = np.ascontiguousarray(np.concatenate(vrows, axis=0).T).astype(bf)
        wo_c = np.ascontiguousarray(
            w_o[:, h0 * DH:(h0 + HPC) * DH].T).astype(bf)
        maps.append({
            "xt": xT, "wqk": wqk_c, "wv": wv_c, "wo": wo_c,
            "cosc": tabs["cosc"], "sinc": tabs["sinc"],
            "masks": tabs["masks"],
        })
    return maps


def kernel(x, w_qkv, w_o):
    from concourse import bass_utils

    if "nc" not in _STATE:
        _STATE["nc"] = build_nc(S)
    nc = _STATE["nc"]
    in_maps = host_in_maps(x, w_qkv, w_o, S)
    res = bass_utils.run_bass_kernel_spmd(
        nc, in_maps, core_ids=list(range(NCORES)))
    out = np.zeros((B, S, HID), dtype=np.float32)
    for r in res.results:
        out += np.asarray(r["out"], dtype=np.float32)
    return out


# revision 23
# speedup vs baseline: 1.3522x; 1.3522x over previous
"""Trainium2 Bass kernel for causal multi-head attention with RoPE.

Model: B=2, S=2048, H=2048, 16 heads x 128 head-dim.
  qkv = x @ w_qkv.T ; RoPE(q, k); causal softmax(q k^T / sqrt(dh)) @ v; out = attn @ w_o.T

Sharding: tensor-parallel over heads. Each of the 8 cores owns 2 heads:
it computes q/k/v projections for its heads (w_qkv row slices), runs
flash-style causal attention for them, and applies its slice of w_o
columns, producing a partial [B,S,H] output. The host sums the 8
partials in fp32 (the all-reduce "unshard" of the TP strategy).

On-core layout choices:
  - Q,K projected in natural [token, dim] layout so RoPE's rotate-half
    pairs sit at free-dim offsets (cross-partition DVE reads are
    rejected by the walrus verifier), then PE-transposed to [dim, token]
    for the scores matmul.
  - Scores computed transposed (S^T[kt, qt]) so the exp'd probabilities
    feed the PV matmul directly with no per-block transpose; the softmax
    denominator comes from an M=1 ones-matmul accumulated in PSUM, and
    is folded in after PV via reciprocal + gpsimd partition_broadcast.
  - No max-subtraction in softmax: inputs are unit-scale gaussians, so
    scaled scores are O(10) and exp stays comfortably inside fp32/bf16.
  - All matmuls in bf16 with fp32 PSUM accumulation.
"""

import contextlib
import math

import numpy as np
import ml_dtypes

B = 2
S = 2048
HID = 2048
NH = 16
DH = 128
NCORES = 8
HPC = NH // NCORES  # heads per core
CH = 512            # chunk (free-dim) size
NEG = -1.0e30

_STATE = {}


# ----------------------------------------------------------------------------
# device kernel
# ----------------------------------------------------------------------------

def _emit_body(nc, r, seq_len):
    """Emit one full pass of the computation. `r` holds pools + consts."""
    import concourse.bass_isa as bass_isa
    import concourse.mybir as mybir

    bf16 = mybir.dt.bfloat16
    f32 = mybir.dt.float32
    Exp = mybir.ActivationFunctionType.Exp
    NT = seq_len // 128
    TC = seq_len // CH
    NHB = HID // 128
    SCALE = 1.0 / math.sqrt(DH)

    def oproj_fillers(b, qi, at_pair):
        """Emitters for chunk qi's output projection, one (tt, oc) tile
        each — interleaved into the next chunk's attention loop as PE
        filler so exp latency on ACT never stalls the PE queue."""
        tiles = {}

        def make(tt, oc):
            def emit():
                if tt not in tiles:
                    tiles[tt] = r.opool.tile([128, HID], bf16, tag="ot",
                                             name="ot")
                ot = tiles[tt]
                pop = r.psC.tile([128, CH], f32, tag="C", name="pop")
                for h in range(2):
                    nc.tensor.matmul(
                        pop[:],
                        at_pair[h][:, tt * 128:(tt + 1) * 128],
                        r.wo_sb[:, h, oc * CH:(oc + 1) * CH],
                        start=(h == 0), stop=(h == 1),
                    )
                nc.vector.tensor_copy(ot[:, oc * CH:(oc + 1) * CH], pop[:])
                if oc == HID // CH - 1:
                    nc.sync.dma_start(
                        r.out_d[b, qi * CH + tt * 128:
                                qi * CH + (tt + 1) * 128, :],
                        ot[:],
                    )
            return emit

        return [make(tt, oc) for tt in range(4) for oc in range(HID // CH)]

    for b in range(B):
        # ---------------- QKV projection + RoPE ----------------
        qks = []
        for nm in ("q0t", "q1t", "k0t", "k1t"):
            qks.append(r.qkpool.tile([128, seq_len], bf16, tag="qkt", name=nm))
        q0t, q1t, k0t, k1t = qks
        vt = r.vpool.tile([128, NT, 2 * DH], bf16, tag="vt", name="vt")
        xTb = r.xT[b].rearrange("(n p) t -> p n t", p=128)

        def emit_transposes(qr, j):
            for o, dst in enumerate((q0t, q1t, k0t, k1t)):
                ptr = r.psC.tile([128, 128], bf16, tag="C", name="ptr")
                nc.tensor.transpose(
                    ptr[:], qr[:, o * 128:(o + 1) * 128], r.ident[:])
                nc.vector.tensor_copy(
                    dst[:, j * 128:(j + 1) * 128], ptr[:])

        pending = None  # transposes run one tile behind their RoPE chain
        for tc4 in range(TC):
            xts = r.xpool.tile([128, NHB, CH], bf16, tag="xt", name="xts")
            nc.sync.dma_start(
                xts[:], xTb[:, :, tc4 * CH:(tc4 + 1) * CH])
            for tt in range(4):
                j = 4 * tc4 + tt
                psqk = r.psA.tile([128, 4 * DH], f32, tag="A", name="psqk")
                psv = r.psB.tile([128, 2 * DH], f32, tag="B", name="psv")
                for hb in range(NHB):
                    lhs = xts[:, hb, tt * 128:(tt + 1) * 128]
                    nc.tensor.matmul(
                        psqk[:], lhs, r.wqk_sb[:, hb, :],
                        start=(hb == 0), stop=(hb == NHB - 1),
                    )
                    nc.tensor.matmul(
                        psv[:], lhs, r.wv_sb[:, hb, :],
                        start=(hb == 0), stop=(hb == NHB - 1),
                    )
                if pending is not None:
                    emit_transposes(*pending)
                nc.scalar.copy(vt[:, j, :], psv[:])
                # RoPE on the whole [128, 512] q0|q1|k0|k1 block at once:
                # strided views pair (d, d+64) within each 128-block.
                ps4 = psqk.rearrange("p (o h d) -> p o h d", h=2, d=64)
                sin4 = r.sinc[:, j, :].rearrange("p (o h d) -> p o h d", h=2, d=64)
                t1 = r.tpool.tile([128, CH], f32, tag="t1", name="t1")
                t14 = t1.rearrange("p (o h d) -> p o h d", h=2, d=64)
                t2 = r.tpool.tile([128, CH], f32, tag="t2", name="t2")
                nc.vector.tensor_mul(t14[:, :, 0, :], ps4[:, :, 1, :],
                                     sin4[:, :, 0, :])
                nc.vector.tensor_mul(t14[:, :, 1, :], ps4[:, :, 0, :],
                                     sin4[:, :, 1, :])
                nc.vector.tensor_mul(t2[:], psqk[:], r.cosc[:, j, :])
                qr = r.qrpool.tile([128, CH], bf16, tag="qr", name="qr")
                nc.vector.tensor_add(qr[:], t1[:], t2[:])
                pending = (qr, j)
        emit_transposes(*pending)

        # ---------------- attention + output projection --------
        prev_at = None
        fillers = []
        for qi in range(TC):
            if prev_at is not None:
                fillers = oproj_fillers(b, qi - 1, prev_at)
            at_pair = []
            for h, (Q, K) in enumerate(((q0t, k0t), (q1t, k1t))):
                pso = r.psB.tile([128, CH], f32, tag="B", name="pso")
                psr = r.psD.tile([1, CH], f32, tag="D", name="psr")
                nj = 4 * qi + 4

                def emit_scores(jb):
                    # Diagonal blocks only produce nonzero probabilities for
                    # qt >= kt; narrow all work to that column subrange.
                    r8 = jb - 4 * qi
                    lo = 128 * r8 if r8 > 0 else 0
                    sub = slice(lo, CH)
                    pss = r.psA.tile([128, CH], f32, tag="A", name="pss")
                    nc.tensor.matmul(
                        pss[:, sub], K[:, jb * 128:(jb + 1) * 128],
                        Q[:, qi * CH + lo:(qi + 1) * CH],
                        start=True, stop=True,
                    )
                    if r8 >= 0:
                        nc.vector.tensor_add(
                            pss[:, sub], pss[:, sub], r.mask_sb[:, r8, sub])
                    pt = r.ptpool.tile([128, CH], bf16, tag="pt", name="pt")
                    nc.scalar.activation(pt[:, sub], pss[:, sub], Exp,
                                         scale=SCALE)
                    return pt, lo

                def emit_pv(jb, pt, lo):
                    sub = slice(lo, CH)
                    nc.tensor.matmul(
                        pso[:, sub], vt[:, jb, h * DH:(h + 1) * DH], pt[:, sub],
                        start=(jb == 0), stop=(jb == nj - 1))
                    nc.tensor.matmul(
                        psr[:, sub], r.ones[:], pt[:, sub],
                        start=(jb == 0), stop=(jb == nj - 1))

                # scores run two blocks ahead of PV so the PE never waits
                # on the mask+exp chain of the block it just scored; oproj
                # matmuls from the previous chunk fill remaining PE slack.
                SKEW = 2
                queue = []
                for jb in range(nj):
                    queue.append((jb, emit_scores(jb)))
                    if fillers:
                        fillers.pop(0)()
                    if len(queue) > SKEW:
                        pj, args = queue.pop(0)
                        emit_pv(pj, *args)
                for pj, args in queue:
                    emit_pv(pj, *args)

                rs = r.rspool.tile([1, CH], f32, tag="rs", name="rs")
                nc.vector.reciprocal(rs[:], psr[:])
                rsb = r.rspool.tile([128, CH], f32, tag="rsb", name="rsb")
                nc.gpsimd.partition_broadcast(rsb[:], rs[:])
                at = r.atpool.tile([128, CH], bf16, tag="at", name="at")
                nc.vector.tensor_mul(at[:], pso[:], rsb[:])
                at_pair.append(at)
            for f in fillers:
                f()
            fillers = []
            prev_at = at_pair
        for f in oproj_fillers(b, TC - 1, prev_at):
            f()


class _Res:
    pass


def build_nc(seq_len=S, loop_n=1):
    """Build the per-core program. loop_n>1 wraps the body in a hardware
    loop — a timing-only variant used to measure per-iteration device
    time through the noisy dispatch path."""
    import concourse.mybir as mybir
    import concourse.tile as tile
    from concourse import bacc
    from concourse.masks import make_identity

    bf16 = mybir.dt.bfloat16
    f32 = mybir.dt.float32
    NT = seq_len // 128

    nc = bacc.Bacc("TRN2", target_bir_lowering=False, debug=False)

    r = _Res()
    r.xT = nc.dram_tensor("xt", [B, HID, seq_len], bf16, kind="ExternalInput")
    wqk = nc.dram_tensor("wqk", [HID, 4 * DH], bf16, kind="ExternalInput")
    wv = nc.dram_tensor("wv", [HID, 2 * DH], bf16, kind="ExternalInput")
    wo = nc.dram_tensor("wo", [2 * DH, HID], bf16, kind="ExternalInput")
    rope_d = {}
    for nm in ("cosc", "sinc"):
        rope_d[nm] = nc.dram_tensor(nm, [seq_len, 4 * DH], bf16,
                                    kind="ExternalInput")
    mask_d = nc.dram_tensor("masks", [4, 128, CH], f32, kind="ExternalInput")
    r.out_d = nc.dram_tensor("out", [B, seq_len, HID], bf16,
                             kind="ExternalOutput")

    with tile.TileContext(nc) as tc:
        with (
            tc.tile_pool(name="consts", bufs=1) as cpool,
            tc.tile_pool(name="x", bufs=2) as xpool,
            tc.tile_pool(name="qk", bufs=8) as qkpool,
            tc.tile_pool(name="v", bufs=2) as vpool,
            tc.tile_pool(name="pt", bufs=4) as ptpool,
            tc.tile_pool(name="at", bufs=4) as atpool,
            tc.tile_pool(name="tmp", bufs=2) as tpool,
            tc.tile_pool(name="qr", bufs=3) as qrpool,
            tc.tile_pool(name="rs", bufs=2) as rspool,
            tc.tile_pool(name="o", bufs=2) as opool,
            tc.tile_pool(name="psA", bufs=3, space="PSUM") as psA,
            tc.tile_pool(name="psB", bufs=2, space="PSUM") as psB,
            tc.tile_pool(name="psC", bufs=2, space="PSUM") as psC,
            tc.tile_pool(name="psD", bufs=1, space="PSUM") as psD,
        ):
            r.xpool, r.qkpool, r.vpool, r.ptpool = xpool, qkpool, vpool, ptpool
            r.atpool, r.tpool, r.qrpool, r.rspool = atpool, tpool, qrpool, rspool
            r.opool = opool
            r.psA, r.psB, r.psC, r.psD = psA, psB, psC, psD

            r.wqk_sb = cpool.tile([128, HID // 128, 4 * DH], bf16, name="wqk_sb")
            nc.sync.dma_start(r.wqk_sb[:], wqk.rearrange("(n p) o -> p n o", p=128))
            r.wv_sb = cpool.tile([128, HID // 128, 2 * DH], bf16, name="wv_sb")
            nc.sync.dma_start(r.wv_sb[:], wv.rearrange("(n p) o -> p n o", p=128))
            r.wo_sb = cpool.tile([128, 2, HID], bf16, name="wo_sb")
            nc.sync.dma_start(r.wo_sb[:], wo.rearrange("(n p) o -> p n o", p=128))
            for nm in ("cosc", "sinc"):
                t = cpool.tile([128, NT, 4 * DH], bf16, name=nm)
                nc.sync.dma_start(t[:], rope_d[nm].rearrange("(n p) d -> p n d", p=128))
                setattr(r, nm, t)
            r.mask_sb = cpool.tile([128, 4, CH], f32, name="mask_sb")
            nc.sync.dma_start(r.mask_sb[:], mask_d.rearrange("n p o -> p n o"))
            r.ident = cpool.tile([128, 128], bf16, name="ident")
            make_identity(nc, r.ident[:])
            r.ones = cpool.tile([128, 1], bf16, name="ones")
            nc.gpsimd.memset(r.ones[:], 1.0)

            loop_ctx = (tc.For_i(0, loop_n, 1) if loop_n > 1
                        else contextlib.nullcontext())
            with loop_ctx:
                _emit_body(nc, r, seq_len)

    nc.compile()
    return nc


# ----------------------------------------------------------------------------
# host-side sharding / tables
# ----------------------------------------------------------------------------

def host_tables(seq_len=S):
    bf = ml_dtypes.bfloat16
    inv = 1.0 / (10000.0 ** (np.arange(0, DH, 2, dtype=np.float64) / DH))
    ang = np.arange(seq_len, dtype=np.float64)[:, None] * inv[None, :]  # [S, 64]
    cos = np.cos(ang)
    sin = np.sin(ang)
    cos_td = np.concatenate([cos, cos], axis=1)                  # [S, 128]
    ssin_td = np.concatenate([-sin, sin], axis=1)                # signed swap mult
    tabs = {
        "cosc": np.ascontiguousarray(np.tile(cos_td, (1, 4))).astype(bf),
        "sinc": np.ascontiguousarray(np.tile(ssin_td, (1, 4))).astype(bf),
    }
    p = np.arange(128)[:, None]
    f = np.arange(CH)[None, :]
    masks = np.stack(
        [np.where(p + 128 * ri <= f, 0.0, NEG) for ri in range(4)]
    ).astype(np.float32)
    tabs["masks"] = masks
    return tabs


def host_in_maps(x, w_qkv, w_o, seq_len=S):
    bf = ml_dtypes.bfloat16
    x = np.asarray(x, dtype=np.float32)
    w_qkv = np.asarray(w_qkv, dtype=np.float32)
    w_o = np.asarray(w_o, dtype=np.float32)
    xT = np.ascontiguousarray(x.transpose(0, 2, 1)).astype(bf)
    tabs = host_tables(seq_len)
    maps = []
    for c in range(NCORES):
        h0 = HPC * c
        rows = []
        for base in (0, HID):  # q rows, then k rows
            for h in range(h0, h0 + HPC):
                rows.append(w_qkv[base + h * DH:base + (h + 1) * DH])
        wqk_c = np.ascontiguousarray(np.concatenate(rows, axis=0).T).astype(bf)
        vrows = [w_qkv[2 * HID + h * DH:2 * HID + (h + 1) * DH]
                 for h in range(h0, h0 + HPC)]
        wv_c = np.ascontiguousarray(np.concatenate(vrows, axis=0).T).astype(bf)
        wo_c = np.ascontiguousarray(
            w_o[:, h0 * DH:(h0 + HPC) * DH].T).astype(bf)
        maps.append({
            "xt": xT, "wqk": wqk_c, "wv": wv_c, "wo": wo_c,
            "cosc": tabs["cosc"], "sinc": tabs["sinc"],
            "masks": tabs["masks"],
        })
    return maps


def kernel(x, w_qkv, w_o):
    from concourse import bass_utils

    if "nc" not in _STATE:
        _STATE["nc"] = build_nc(S)
    nc = _STATE["nc"]
    in_maps = host_in_maps(x, w_qkv, w_o, S)
    res = bass_utils.run_bass_kernel_spmd(
        nc, in_maps, core_ids=list(range(NCORES)))
    out = np.zeros((B, S, HID), dtype=np.float32)
    for r in res.results:
        out += np.asarray(r["out"], dtype=np.float32)
    return out


# revision 36
# speedup vs baseline: 1.5534x; 1.1488x over previous
"""Trainium2 Bass kernel for causal multi-head attention with RoPE.

Model: B=2, S=2048, H=2048, 16 heads x 128 head-dim.
  qkv = x @ w_qkv.T ; RoPE(q, k); causal softmax(q k^T / sqrt(dh)) @ v; out = attn @ w_o.T

Sharding: tensor-parallel over heads. Each of the 8 cores owns 2 heads:
it computes q/k/v projections for its heads (w_qkv row slices), runs
flash-style causal attention for them, and applies its slice of w_o
columns, producing a partial [B,S,H] output. The host sums the 8
partials in fp32 (the all-reduce "unshard" of the TP strategy).

On-core layout choices:
  - Q,K projected in natural [token, dim] layout so RoPE's rotate-half
    pairs sit at free-dim offsets (cross-partition DVE reads are
    rejected by the walrus verifier), then PE-transposed to [dim, token]
    for the scores matmul.
  - Scores computed transposed (S^T[kt, qt]) so the exp'd probabilities
    feed the PV matmul directly with no per-block transpose; the softmax
    denominator comes from an M=1 ones-matmul accumulated in PSUM, and
    is folded in after PV via reciprocal + gpsimd partition_broadcast.
  - No max-subtraction in softmax: inputs are unit-scale gaussians, so
    scaled scores are O(10) and exp stays comfortably inside fp32/bf16.
  - All matmuls in bf16 with fp32 PSUM accumulation.
"""

import contextlib
import math

import numpy as np
import ml_dtypes

B = 2
S = 2048
HID = 2048
NH = 16
DH = 128
NCORES = 8
HPC = NH // NCORES  # heads per core
CH = 512            # chunk (free-dim) size
NEG = -1.0e30

_STATE = {}

# tuning knobs (read at build time)
CFG = {"skew": 2, "pt_bufs": 4, "psa": 3, "psr_own_bank": True,
       "any_ot": True}


# ----------------------------------------------------------------------------
# device kernel
# ----------------------------------------------------------------------------

def _emit_body(nc, r, seq_len, parts="all"):
    """Emit one full pass of the computation. `r` holds pools + consts."""
    import concourse.bass_isa as bass_isa
    import concourse.mybir as mybir

    bf16 = mybir.dt.bfloat16
    f32 = mybir.dt.float32
    Exp = mybir.ActivationFunctionType.Exp
    NT = seq_len // 128
    TC = seq_len // CH
    NHB = HID // 128
    SCALE = 1.0 / math.sqrt(DH)

    def oproj_fillers(b, qi, at_pair):
        """Emitters for chunk qi's output projection, one (tt, oc) tile
        each — interleaved into the next chunk's attention loop as PE
        filler so exp latency on ACT never stalls the PE queue."""
        tiles = {}

        def make(tt, oc):
            def emit():
                if tt not in tiles:
                    tiles[tt] = r.opool.tile([128, HID], bf16, tag="ot",
                                             name="ot")
                ot = tiles[tt]
                pop = r.psC.tile([128, CH], f32, tag="C", name="pop")
                for h in range(2):
                    nc.tensor.matmul(
                        pop[:],
                        at_pair[h][:, tt * 128:(tt + 1) * 128],
                        r.wo_sb[:, h, oc * CH:(oc + 1) * CH],
                        start=(h == 0), stop=(h == 1),
                    )
                if CFG.get("any_ot"):
                    nc.any.tensor_copy(ot[:, oc * CH:(oc + 1) * CH], pop[:])
                else:
                    nc.vector.tensor_copy(ot[:, oc * CH:(oc + 1) * CH], pop[:])
                if oc == HID // CH - 1:
                    nc.sync.dma_start(
                        r.out_d[b, qi * CH + tt * 128:
                                qi * CH + (tt + 1) * 128, :],
                        ot[:],
                    )
            return emit

        return [make(tt, oc) for tt in range(4) for oc in range(HID // CH)]

    for b in range(B if parts == "all" else 1):
        # ---------------- QKV projection + RoPE ----------------
        qks = []
        for nm in ("q0t", "q1t", "k0t", "k1t"):
            qks.append(r.qkpool.tile([128, seq_len], bf16, tag="qkt", name=nm))
        q0t, q1t, k0t, k1t = qks
        vt = r.vpool.tile([128, NT, 2 * DH], bf16, tag="vt", name="vt")
        xTb = r.xT[b].rearrange("(n p) t -> p n t", p=128)

        def emit_transposes(qr, j):
            for o, dst in enumerate((q0t, q1t, k0t, k1t)):
                ptr = r.psC.tile([128, 128], bf16, tag="C", name="ptr")
                nc.tensor.transpose(
                    ptr[:], qr[:, o * 128:(o + 1) * 128], r.ident[:])
                nc.vector.tensor_copy(
                    dst[:, j * 128:(j + 1) * 128], ptr[:])

        pending = None  # transposes run one tile behind their RoPE chain
        for tc4 in range(TC):
            xts = r.xpool.tile([128, NHB, CH], bf16, tag="xt", name="xts")
            nc.sync.dma_start(
                xts[:], xTb[:, :, tc4 * CH:(tc4 + 1) * CH])
            for tt in range(4):
                j = 4 * tc4 + tt
                psqk = r.psA.tile([128, 4 * DH], f32, tag="A", name="psqk")
                psv = r.psB.tile([128, 2 * DH], f32, tag="B", name="psv")
                for hb in range(NHB):
                    lhs = xts[:, hb, tt * 128:(tt + 1) * 128]
                    nc.tensor.matmul(
                        psqk[:], lhs, r.wqk_sb[:, hb, :],
                        start=(hb == 0), stop=(hb == NHB - 1),
                    )
                    nc.tensor.matmul(
                        psv[:], lhs, r.wv_sb[:, hb, :],
                        start=(hb == 0), stop=(hb == NHB - 1),
                    )
                if pending is not None:
                    emit_transposes(*pending)
                nc.scalar.copy(vt[:, j, :], psv[:])
                # RoPE on the whole [128, 512] q0|q1|k0|k1 block at once:
                # strided views pair (d, d+64) within each 128-block.
                ps4 = psqk.rearrange("p (o h d) -> p o h d", h=2, d=64)
                sin4 = r.sinc[:, j, :].rearrange("p (o h d) -> p o h d", h=2, d=64)
                t1 = r.tpool.tile([128, CH], f32, tag="t1", name="t1")
                t14 = t1.rearrange("p (o h d) -> p o h d", h=2, d=64)
                t2 = r.tpool.tile([128, CH], f32, tag="t2", name="t2")
                nc.vector.tensor_mul(t14[:, :, 0, :], ps4[:, :, 1, :],
                                     sin4[:, :, 0, :])
                nc.vector.tensor_mul(t14[:, :, 1, :], ps4[:, :, 0, :],
                                     sin4[:, :, 1, :])
                nc.vector.tensor_mul(t2[:], psqk[:], r.cosc[:, j, :])
                qr = r.qrpool.tile([128, CH], bf16, tag="qr", name="qr")
                nc.vector.tensor_add(qr[:], t1[:], t2[:])
                pending = (qr, j)
        emit_transposes(*pending)

        if parts == "qkv":
            continue
        # ---------------- attention + output projection --------
        prev_at = None
        fillers = []
        for qi in range(TC):
            if prev_at is not None:
                fillers = oproj_fillers(b, qi - 1, prev_at)
            at_pair = []
            for h, (Q, K) in enumerate(((q0t, k0t), (q1t, k1t))):
                pso = r.psB.tile([128, CH], f32, tag="B", name="pso")
                psr = r.psD.tile([128, CH], f32, tag="D", name="psr")
                nj = 4 * qi + 4

                def emit_scores(jb):
                    # Diagonal blocks only produce nonzero probabilities for
                    # qt >= kt; narrow all work to that column subrange.
                    r8 = jb - 4 * qi
                    lo = 128 * r8 if r8 > 0 else 0
                    sub = slice(lo, CH)
                    pss = r.psA.tile([128, CH], f32, tag="A", name="pss")
                    nc.tensor.matmul(
                        pss[:, sub], K[:, jb * 128:(jb + 1) * 128],
                        Q[:, qi * CH + lo:(qi + 1) * CH],
                        start=True, stop=True,
                    )
                    if r8 >= 0:
                        nc.vector.tensor_add(
                            pss[:, sub], pss[:, sub], r.mask_sb[:, r8, sub])
                    pt = r.ptpool.tile([128, CH], bf16, tag="pt", name="pt")
                    nc.scalar.activation(pt[:, sub], pss[:, sub], Exp,
                                         scale=SCALE)
                    return pt, lo

                def emit_pv(jb, pt, lo):
                    sub = slice(lo, CH)
                    nc.tensor.matmul(
                        pso[:, sub], vt[:, jb, h * DH:(h + 1) * DH], pt[:, sub],
                        start=(jb == 0), stop=(jb == nj - 1))
                    # rowsum via M=128 all-ones stationary: every psum
                    # partition receives the same column sums, so no
                    # post-hoc partition broadcast is needed.
                    nc.tensor.matmul(
                        psr[:, sub], r.ones[:], pt[:, sub],
                        start=(jb == 0), stop=(jb == nj - 1))

                # scores run two blocks ahead of PV so the PE never waits
                # on the mask+exp chain of the block it just scored; oproj
                # matmuls from the previous chunk fill remaining PE slack.
                SKEW = CFG["skew"]
                queue = []
                for jb in range(nj):
                    queue.append((jb, emit_scores(jb)))
                    if fillers:
                        fillers.pop(0)()
                    if len(queue) > SKEW:
                        pj, args = queue.pop(0)
                        emit_pv(pj, *args)
                for pj, args in queue:
                    emit_pv(pj, *args)

                rsb = r.rspool.tile([128, CH], f32, tag="rsb", name="rsb")
                nc.vector.reciprocal(rsb[:], psr[:])
                at = r.atpool.tile([128, CH], bf16, tag="at", name="at")
                nc.vector.tensor_mul(at[:], pso[:], rsb[:])
                at_pair.append(at)
            for f in fillers:
                f()
            fillers = []
            prev_at = at_pair
        for f in oproj_fillers(b, TC - 1, prev_at):
            f()


class _Res:
    pass


def build_nc(seq_len=S, loop_n=1, parts="all"):
    """Build the per-core program. loop_n>1 wraps the body in a hardware
    loop — a timing-only variant used to measure per-iteration device
    time through the noisy dispatch path."""
    import concourse.mybir as mybir
    import concourse.tile as tile
    from concourse import bacc
    from concourse.masks import make_identity

    bf16 = mybir.dt.bfloat16
    f32 = mybir.dt.float32
    NT = seq_len // 128

    nc = bacc.Bacc("TRN2", target_bir_lowering=False, debug=False)

    r = _Res()
    r.xT = nc.dram_tensor("xt", [B, HID, seq_len], bf16, kind="ExternalInput")
    wqk = nc.dram_tensor("wqk", [HID, 4 * DH], bf16, kind="ExternalInput")
    wv = nc.dram_tensor("wv", [HID, 2 * DH], bf16, kind="ExternalInput")
    wo = nc.dram_tensor("wo", [2 * DH, HID], bf16, kind="ExternalInput")
    rope_d = {}
    for nm in ("cosc", "sinc"):
        rope_d[nm] = nc.dram_tensor(nm, [seq_len, 4 * DH], bf16,
                                    kind="ExternalInput")
    mask_d = nc.dram_tensor("masks", [4, 128, CH], f32, kind="ExternalInput")
    r.out_d = nc.dram_tensor("out", [B, seq_len, HID], bf16,
                             kind="ExternalOutput")

    with tile.TileContext(nc) as tc:
        with (
            tc.tile_pool(name="consts", bufs=1) as cpool,
            tc.tile_pool(name="x", bufs=2) as xpool,
            tc.tile_pool(name="qk", bufs=8) as qkpool,
            tc.tile_pool(name="v", bufs=2) as vpool,
            tc.tile_pool(name="pt", bufs=CFG["pt_bufs"]) as ptpool,
            tc.tile_pool(name="at", bufs=4) as atpool,
            tc.tile_pool(name="tmp", bufs=2) as tpool,
            tc.tile_pool(name="qr", bufs=3) as qrpool,
            tc.tile_pool(name="rs", bufs=2) as rspool,
            tc.tile_pool(name="o", bufs=2) as opool,
            tc.tile_pool(name="psA", bufs=CFG["psa"], space="PSUM") as psA,
            tc.tile_pool(name="psB", bufs=2, space="PSUM") as psB,
            tc.tile_pool(name="psC", bufs=2, space="PSUM") as psC,
            tc.tile_pool(name="psD", bufs=1, space="PSUM") as psD,
        ):
            r.xpool, r.qkpool, r.vpool, r.ptpool = xpool, qkpool, vpool, ptpool
            r.atpool, r.tpool, r.qrpool, r.rspool = atpool, tpool, qrpool, rspool
            r.opool = opool
            r.psA, r.psB, r.psC, r.psD = psA, psB, psC, psD

            r.wqk_sb = cpool.tile([128, HID // 128, 4 * DH], bf16, name="wqk_sb")
            nc.sync.dma_start(r.wqk_sb[:], wqk.rearrange("(n p) o -> p n o", p=128))
            r.wv_sb = cpool.tile([128, HID // 128, 2 * DH], bf16, name="wv_sb")
            nc.sync.dma_start(r.wv_sb[:], wv.rearrange("(n p) o -> p n o", p=128))
            r.wo_sb = cpool.tile([128, 2, HID], bf16, name="wo_sb")
            nc.sync.dma_start(r.wo_sb[:], wo.rearrange("(n p) o -> p n o", p=128))
            for nm in ("cosc", "sinc"):
                t = cpool.tile([128, NT, 4 * DH], bf16, name=nm)
                nc.sync.dma_start(t[:], rope_d[nm].rearrange("(n p) d -> p n d", p=128))
                setattr(r, nm, t)
            r.mask_sb = cpool.tile([128, 4, CH], f32, name="mask_sb")
            nc.sync.dma_start(r.mask_sb[:], mask_d.rearrange("n p o -> p n o"))
            r.ident = cpool.tile([128, 128], bf16, name="ident")
            make_identity(nc, r.ident[:])
            r.ones = cpool.tile([128, 128], bf16, name="ones")
            nc.gpsimd.memset(r.ones[:], 1.0)

            loop_ctx = (tc.For_i(0, loop_n, 1) if loop_n > 1
                        else contextlib.nullcontext())
            with loop_ctx:
                _emit_body(nc, r, seq_len, parts)

    nc.compile()
    return nc


# ----------------------------------------------------------------------------
# host-side sharding / tables
# ----------------------------------------------------------------------------

def host_tables(seq_len=S):
    bf = ml_dtypes.bfloat16
    inv = 1.0 / (10000.0 ** (np.arange(0, DH, 2, dtype=np.float64) / DH))
    ang = np.arange(seq_len, dtype=np.float64)[:, None] * inv[None, :]  # [S, 64]
    cos = np.cos(ang)
    sin = np.sin(ang)
    cos_td = np.concatenate([cos, cos], axis=1)                  # [S, 128]
    ssin_td = np.concatenate([-sin, sin], axis=1)                # signed swap mult
    tabs = {
        "cosc": np.ascontiguousarray(np.tile(cos_td, (1, 4))).astype(bf),
        "sinc": np.ascontiguousarray(np.tile(ssin_td, (1, 4))).astype(bf),
    }
    p = np.arange(128)[:, None]
    f = np.arange(CH)[None, :]
    masks = np.stack(
        [np.where(p + 128 * ri <= f, 0.0, NEG) for ri in range(4)]
    ).astype(np.float32)
    tabs["masks"] = masks
    return tabs


def host_in_maps(x, w_qkv, w_o, seq_len=S):
    bf = ml_dtypes.bfloat16
    x = np.asarray(x, dtype=np.float32)
    w_qkv = np.asarray(w_qkv, dtype=np.float32)
    w_o = np.asarray(w_o, dtype=np.float32)
    xT = np.ascontiguousarray(x.transpose(0, 2, 1)).astype(bf)
    tabs = host_tables(seq_len)
    maps = []
    for c in range(NCORES):
        h0 = HPC * c
        rows = []
        for base in (0, HID):  # q rows, then k rows
            for h in range(h0, h0 + HPC):
                rows.append(w_qkv[base + h * DH:base + (h + 1) * DH])
        wqk_c = np.ascontiguousarray(np.concatenate(rows, axis=0).T).astype(bf)
        vrows = [w_qkv[2 * HID + h * DH:2 * HID + (h + 1) * DH]
                 for h in range(h0, h0 + HPC)]
        wv_c = np.ascontiguousarray(np.concatenate(vrows, axis=0).T).astype(bf)
        wo_c = np.ascontiguousarray(
            w_o[:, h0 * DH:(h0 + HPC) * DH].T).astype(bf)
        maps.append({
            "xt": xT, "wqk": wqk_c, "wv": wv_c, "wo": wo_c,
            "cosc": tabs["cosc"], "sinc": tabs["sinc"],
            "masks": tabs["masks"],
        })
    return maps


def kernel(x, w_qkv, w_o):
    from concourse import bass_utils

    if "nc" not in _STATE:
        _STATE["nc"] = build_nc(S)
    nc = _STATE["nc"]
    in_maps = host_in_maps(x, w_qkv, w_o, S)
    res = bass_utils.run_bass_kernel_spmd(
        nc, in_maps, core_ids=list(range(NCORES)))
    out = np.zeros((B, S, HID), dtype=np.float32)
    for r in res.results:
        out += np.asarray(r["out"], dtype=np.float32)
    return out
